# revision 24
# baseline (speedup 1.0000x reference)
"""HTM spatial-pooler kernel for Trainium2 (8 NeuronCores, data-parallel over tokens).

Computes, for x = input_vector reshaped to [4096 tokens, 4096]:
    overlap = x @ C^T               (C = connections [2048, 4096], binary)
    boosted = overlap * boost       (per-column boosting factors)
    masked  = where(boosted >= kth_largest_per_row(boosted, k), boosted, 0)

Strategy per core (512 tokens):
  - Matmul as THREE fp8(e4m3) passes in DoubleRow perf mode (0.5 cycles/row,
    2 contraction sub-tiles per instruction), all accumulating into a single
    PSUM bank per 512-column block. Scale alignment is folded into a single
    resident copy of C at scale 2^-6 (values {0, 2^-6}, exact in e4m3):
        64*x ~ a + b + c,   overlap = (a+b+c) @ (C * 2^-6)
    with a = e4m3(64x), b = e4m3(64x - a), c = e4m3(64x - a - b). Because
    the e4m3 subnormal floor (2^-9) is divided by the C scale, the residual
    is <= 2^-15 in x units — the top-k mask matches the exact fp32 mask
    except for genuinely tied rows, with no DVE combine passes needed.
  - DVE applies boosting per block, then computes the per-row k-th-largest
    via segmented max8/match_replace and masks with a fused
    (boosted >= thr) * boosted scalar_tensor_tensor. Output stored as bf16.
"""
import math

import numpy as np
import ml_dtypes

import concourse.bacc as bacc
import concourse.mybir as mybir
from concourse import tile
from concourse.bass_utils import run_bass_kernel_spmd

FP8 = mybir.dt.float8e4
BF16 = mybir.dt.bfloat16
F32 = mybir.dt.float32
E4 = ml_dtypes.float8_e4m3

N_CORES = 8
TOK_PER_CORE = 512
M_TILES = 4          # 128-token tiles per core
D = 4096             # input size (contraction)
KC2 = D // 256       # 16 double-row contraction chunks
NCOL = 2048          # minicolumns
NCH = NCOL // 512    # 4 psum column chunks

_BUILD_CACHE = {}


def _build(k_active: int):
    nc = bacc.Bacc("TRN2", target_bir_lowering=False)
    # x passes: [m, ks(128), kc2, pair, tok] ; c6: [ks(128), kc2, pair, col]
    xa = nc.dram_tensor("xa", [M_TILES, 128, KC2 * 2 * 128], FP8, kind="ExternalInput")
    xb = nc.dram_tensor("xb", [M_TILES, 128, KC2 * 2 * 128], FP8, kind="ExternalInput")
    xc = nc.dram_tensor("xc", [M_TILES, 128, KC2 * 2 * 128], FP8, kind="ExternalInput")
    c6 = nc.dram_tensor("c6", [128, KC2, 2, NCOL], FP8, kind="ExternalInput")
    bc = nc.dram_tensor("bc", [128, NCOL], F32, kind="ExternalInput")
    out = nc.dram_tensor("out", [M_TILES, 128, NCOL], BF16, kind="ExternalOutput")

    rounds = max(1, math.ceil(k_active / 8))
    t_idx = (k_active - 1) % 8
    DR = mybir.MatmulPerfMode.DoubleRow

    with tile.TileContext(nc) as tc:
        with (
            tc.tile_pool(name="cpool", bufs=1) as cpool,
            tc.tile_pool(name="xpool", bufs=4) as xpool,
            tc.tile_pool(name="psum", bufs=8, space="PSUM") as pspool,
            tc.tile_pool(name="bpool", bufs=2) as bpool,
            tc.tile_pool(name="wpool", bufs=2) as wpool,
            tc.tile_pool(name="opool", bufs=2) as opool,
        ):
            ct = []

            def load_ct(j):
                # two half DMAs (column halves) so delivery granularity
                # matches the warm-up consumption rate
                t = cpool.tile([128, 2, NCOL], FP8, tag=f"c6_{j}")
                nc.sync.dma_start(t[:, :, :NCOL // 2], c6[:, j, :, :NCOL // 2])
                nc.sync.dma_start(t[:, :, NCOL // 2:], c6[:, j, :, NCOL // 2:])
                ct.append(t)

            def xtile(name, dram, m):
                t = xpool.tile([128, KC2, 2, 128], FP8, tag=name)
                nc.sync.dma_start(t[:], dram[m])
                return t

            # DMA issue order (= serialization order on the DMA engines):
            # the six phase-1 x tiles' FIRST halves (j 0-7) go out up front,
            # interleaved with the first C chunks, so all 8 in-flight blocks
            # (psum bufs) have j-outer work as soon as each C chunk lands;
            # the x second halves follow before j=8 is reached; everything
            # is resident well before the second half of the m-tiles.
            xt = [[None] * 3 for _ in range(M_TILES)]
            XD = {("xa", 0): xa, ("xb", 1): xb, ("xc", 2): xc}
            for (name, pi), dram in XD.items():
                for m in (0, 1):
                    xt[m][pi] = xpool.tile(
                        [128, KC2, 2, 128], FP8, tag=name, name=f"{name}{m}")
            HALF = KC2 // 2 * 2 * 128

            def xhalf(m, pi, h):
                dram = (xa, xb, xc)[pi]
                t = xt[m][pi]
                if h == 0:
                    nc.sync.dma_start(t[:, :KC2 // 2], dram[m][:, :HALF])
                else:
                    nc.sync.dma_start(t[:, KC2 // 2:], dram[m][:, HALF:])

            xhalf(0, 0, 0)
            load_ct(0)
            load_ct(1)
            xhalf(1, 0, 0)
            xhalf(0, 1, 0)
            xhalf(1, 1, 0)
            xhalf(0, 2, 0)
            xhalf(1, 2, 0)
            load_ct(2)
            load_ct(3)
            xhalf(0, 0, 1)
            xhalf(1, 0, 1)
            load_ct(4)
            xhalf(0, 1, 1)
            xhalf(1, 1, 1)
            load_ct(5)
            xhalf(0, 2, 1)
            xhalf(1, 2, 1)
            for j in range(6, KC2):
                load_ct(j)
            bc_t = cpool.tile([128, NCOL], F32)
            nc.sync.dma_start(bc_t[:], bc[:])
            for m in (2, 3):
                xt[m][0] = xtile("xa", xa, m)
                xt[m][1] = xtile("xb", xb, m)
                xt[m][2] = xtile("xc", xc, m)

            def matmuls_for(m, n, ps, j_range, pass_range):
                for pi in pass_range:
                    xp = xt[m][pi]
                    for j in j_range:
                        nc.tensor.matmul(
                            ps[:],
                            xp[:, j, :, :],
                            ct[j][:, :, n * 512:(n + 1) * 512],
                            start=(pi == 0 and j == 0),
                            stop=(pi == 2 and j == KC2 - 1),
                            perf_mode=DR,
                        )

            NB = 8 * rounds          # per-block survivors (40 for k=40)
            FW = 3 * NB + 64         # final round width

            def block_rounds(n, cands, cands2):
                # exact top-NB of block n's 64 candidates -> cands2 slice
                # (runs under the next block's matmuls)
                wcb = wpool.tile([128, 64], F32, tag="wcb")
                src = cands[:, n * 64:(n + 1) * 64]
                for r in range(rounds):
                    m8 = cands2[:, n * NB + r * 8:n * NB + (r + 1) * 8]
                    nc.vector.max(m8, src)
                    if r != rounds - 1:
                        nc.vector.match_replace(wcb[:], m8, src, 0.0)
                        src = wcb[:]

            def finish_block(m, n, ps, boosted, cands, cands2):
                # cands2 set (last m-tile): blocks 0..2 reduce to their exact
                # top-NB under the matmuls and the last block's raw segment
                # maxes land directly in the final array, shortening the
                # critical tail chain. cands2 None: plain 256-wide candidates.
                blk = boosted[:, n * 512:(n + 1) * 512]
                nc.vector.tensor_tensor(
                    blk, ps[:], bc_t[:, n * 512:(n + 1) * 512],
                    mybir.AluOpType.mult)
                if k_active <= 48:
                    for s in range(8):
                        dst = (cands[:, (n * 8 + s) * 8:(n * 8 + s + 1) * 8]
                               if (cands2 is None or n < NCH - 1) else
                               cands2[:, 3 * NB + s * 8:3 * NB + (s + 1) * 8])
                        nc.vector.max(
                            dst,
                            boosted[:, (n * 8 + s) * 64:(n * 8 + s + 1) * 64],
                        )
                    if cands2 is not None and n < NCH - 1:
                        block_rounds(n, cands, cands2)

            # Phase 1 (m0+m1, all 8 psum banks): j-outer emission so every
            # arriving C chunk immediately feeds all 8 in-flight blocks.
            row = {}
            for m in (0, 1):
                row[m] = (bpool.tile([128, NCOL], F32, tag="boosted",
                                     name=f"boosted{m}"),
                          wpool.tile([128, 32 * 8], F32, tag="cands",
                                     name=f"cands{m}"),
                          None)
            ps1 = {(m, n): pspool.tile([128, 512], F32, tag="ps",
                                       name=f"ps{m}{n}")
                   for m in (0, 1) for n in range(NCH)}
            # j-outer only over the C-arrival window; then complete
            # blocks one at a time (C resident by then) so they stop
            # staggered and the DVE chains start ~25us in rather than at
            # phase-1's end.
            JW = 9
            for j in range(JW):
                for pi in range(3):
                    for m in (0, 1):
                        for n in range(NCH):
                            matmuls_for(m, n, ps1[(m, n)], [j], [pi])

            def tail_chain(m, boosted, src_c, width):
                if k_active <= 48:
                    # Exact k-th largest of the surviving candidates (a
                    # 64-col segment contributes >8 of the top-k with prob
                    # ~2e-4 per row for k=40), then threshold-mask the row.
                    tops = wpool.tile([128, 8 * rounds], F32, tag="tops")
                    wc = wpool.tile([128, width], F32, tag="wc")
                    src = src_c[:, :width]
                    for r in range(rounds):
                        m8 = tops[:, r * 8:(r + 1) * 8]
                        nc.vector.max(m8, src)
                        if r != rounds - 1:
                            nc.vector.match_replace(wc[:], m8, src, 0.0)
                            src = wc[:]
                    thr = tops[:, (rounds - 1) * 8 + t_idx:
                               (rounds - 1) * 8 + t_idx + 1]
                    # masked = (boosted >= thr) * boosted, fused, in quarters
                    # with the output DMA per quarter (issued from the DVE
                    # queue: no cross-engine hop) to keep the tail short.
                    mbf = opool.tile([128, NCOL], BF16, tag="mbf")
                    edges = (0, 1024, 1536, 1792, 2048)
                    for h in range(4):
                        sl = slice(edges[h], edges[h + 1])
                        nc.vector.scalar_tensor_tensor(
                            mbf[:, sl], boosted[:, sl], thr, boosted[:, sl],
                            mybir.AluOpType.is_ge, mybir.AluOpType.mult)
                        nc.scalar.dma_start(out[m][:, sl], mbf[:, sl])
                else:
                    # Exact full-width chain: zero the top-k in a working
                    # copy, then masked = boosted - working.
                    rem = k_active % 8
                    tops = wpool.tile([128, 8 * rounds], F32, tag="tops")
                    w = wpool.tile([128, NCOL], F32, tag="w")
                    src = boosted
                    for r in range(rounds):
                        m8 = tops[:, r * 8:(r + 1) * 8]
                        nc.vector.max(m8, src[:])
                        if r == rounds - 1 and rem:
                            nc.gpsimd.memset(m8[:, rem:], -1e30)
                        nc.vector.match_replace(w[:], m8, src[:], 0.0)
                        src = w
                    mbf = opool.tile([128, NCOL], BF16, tag="mbf")
                    nc.vector.tensor_tensor(
                        mbf[:], boosted[:], w[:], mybir.AluOpType.subtract)
                    nc.sync.dma_start(out[m], mbf[:])

            for m in (0, 1):
                for n in range(NCH):
                    matmuls_for(m, n, ps1[(m, n)], range(JW, KC2), range(3))
                    finish_block(m, n, ps1[(m, n)], *row[m])
                tail_chain(m, row[m][0], row[m][1], 256)

            # Phase 2 (m2, m3): C fully resident — block-sequential.
            # Only the final m-tile uses the per-block top-NB reduction
            # (shorter tail); earlier tiles keep the cheaper 256-wide rounds.
            for m in (2, 3):
                boosted = bpool.tile([128, NCOL], F32, tag="boosted")
                cands = wpool.tile([128, 32 * 8], F32, tag="cands")
                cands2 = (wpool.tile([128, FW], F32, tag="cands2",
                                     name="cands2")
                          if m == 3 else None)
                last_n = NCH - 1 if m == 3 else NCH
                for n in range(last_n):
                    ps = pspool.tile([128, 512], F32, tag="ps")
                    matmuls_for(m, n, ps, range(KC2), range(3))
                    finish_block(m, n, ps, boosted, cands, cands2)
                if m == 3:
                    # final block in four 128-col sub-blocks so its boost +
                    # segment maxes overlap the remaining sub-blocks' matmuls
                    n = NCH - 1
                    psl = pspool.tile([128, 512], F32, tag="ps", name="psl")
                    for q in range(4):
                        c0 = n * 512 + q * 128
                        psq = psl[:, q * 128:(q + 1) * 128]
                        for pi in range(3):
                            xp = xt[m][pi]
                            for j in range(KC2):
                                nc.tensor.matmul(
                                    psq,
                                    xp[:, j, :, :],
                                    ct[j][:, :, c0:c0 + 128],
                                    start=(pi == 0 and j == 0),
                                    stop=(pi == 2 and j == KC2 - 1),
                                    perf_mode=DR,
                                    skip_group_check=True,
                                )
                        nc.vector.tensor_tensor(
                            boosted[:, c0:c0 + 128], psq,
                            bc_t[:, c0:c0 + 128], mybir.AluOpType.mult)
                        if k_active <= 48:
                            for s2 in range(2):
                                sg = 2 * q + s2
                                nc.vector.max(
                                    cands2[:, 3 * NB + sg * 8:
                                           3 * NB + (sg + 1) * 8],
                                    boosted[:, c0 + s2 * 64:c0 + (s2 + 1) * 64],
                                )
                    tail_chain(m, boosted, cands2, FW)
                else:
                    tail_chain(m, boosted, cands, 256)
    nc.compile()
    return nc


def _get_nc(k_active: int):
    nc = _BUILD_CACHE.get(k_active)
    if nc is None:
        nc = _BUILD_CACHE[k_active] = _build(k_active)
    return nc


def _fp8_split3(x):
    """x (f32, [0,1)) -> (a, b, c) e4m3 with (a + b + c)/64 ~ x
    (residual <= 2^-15)."""
    a = (x * 64.0).astype(E4)
    r1 = x * 64.0 - a.astype(np.float32)
    b = r1.astype(E4)
    r2 = r1 - b.astype(np.float32)
    c = r2.astype(E4)
    return a, b, c


def kernel(input_vector, connections, boosting_factors, num_active):
    x = np.ascontiguousarray(input_vector, dtype=np.float32).reshape(-1, D)
    b = np.ascontiguousarray(boosting_factors, dtype=np.float32)
    k = min(int(num_active), NCOL)
    n_tok = x.shape[0]
    assert n_tok == N_CORES * TOK_PER_CORE, n_tok

    nc = _get_nc(k)

    # x^T laid out as [core, m, ks(part), kc2, pair, tok]
    xt = np.ascontiguousarray(x.T)                         # [D, n_tok]
    xt = xt.reshape(KC2, 2, 128, N_CORES, M_TILES, 128)    # [j, i, ks, core, m, t]
    xt = xt.transpose(3, 4, 2, 0, 1, 5)                    # [core, m, ks, j, i, t]
    xt = np.ascontiguousarray(xt).reshape(N_CORES, M_TILES, 128, KC2 * 2 * 128)
    xa, xb, xc = _fp8_split3(xt)

    # C^T laid out as [ks(part), kc2, pair, col]; {0, 2^-6} exact in e4m3
    ct = np.ascontiguousarray(connections.T, dtype=np.float32)  # [D, NCOL]
    ct = ct.reshape(KC2, 2, 128, NCOL).transpose(2, 0, 1, 3)
    c6 = (np.ascontiguousarray(ct) * 0.015625).astype(E4)

    bcast = np.ascontiguousarray(np.broadcast_to(b, (128, NCOL)))

    in_maps = [
        {"xa": xa[cidx], "xb": xb[cidx], "xc": xc[cidx], "c6": c6, "bc": bcast}
        for cidx in range(N_CORES)
    ]
    res = run_bass_kernel_spmd(nc, in_maps, core_ids=list(range(N_CORES)))
    outs = [r["out"].astype(np.float32).reshape(TOK_PER_CORE, NCOL)
            for r in res.results]
    full = np.concatenate(outs, axis=0)
    return full.reshape(input_vector.shape[0], input_vector.shape[1], NCOL)


# revision 25
# speedup vs baseline: 1.0128x; 1.0128x over previous
"""HTM spatial-pooler kernel for Trainium2 (8 NeuronCores, data-parallel over tokens).

Computes, for x = input_vector reshaped to [4096 tokens, 4096]:
    overlap = x @ C^T               (C = connections [2048, 4096], binary)
    boosted = overlap * boost       (per-column boosting factors)
    masked  = where(boosted >= kth_largest_per_row(boosted, k), boosted, 0)

Strategy per core (512 tokens):
  - Matmul as THREE fp8(e4m3) passes in DoubleRow perf mode (0.5 cycles/row,
    2 contraction sub-tiles per instruction), all accumulating into a single
    PSUM bank per 512-column block. Scale alignment is folded into a single
    resident copy of C at scale 2^-6 (values {0, 2^-6}, exact in e4m3):
        64*x ~ a + b + c,   overlap = (a+b+c) @ (C * 2^-6)
    with a = e4m3(64x), b = e4m3(64x - a), c = e4m3(64x - a - b). Because
    the e4m3 subnormal floor (2^-9) is divided by the C scale, the residual
    is <= 2^-15 in x units — the top-k mask matches the exact fp32 mask
    except for genuinely tied rows, with no DVE combine passes needed.
  - DVE applies boosting per block, then computes the per-row k-th-largest
    via segmented max8/match_replace and masks with a fused
    (boosted >= thr) * boosted scalar_tensor_tensor. Output stored as bf16.
"""
import math

import numpy as np
import ml_dtypes

import concourse.bacc as bacc
import concourse.mybir as mybir
from concourse import tile
from concourse.bass_utils import run_bass_kernel_spmd

FP8 = mybir.dt.float8e4
BF16 = mybir.dt.bfloat16
F32 = mybir.dt.float32
E4 = ml_dtypes.float8_e4m3

N_CORES = 8
TOK_PER_CORE = 512
M_TILES = 4          # 128-token tiles per core
D = 4096             # input size (contraction)
KC2 = D // 256       # 16 double-row contraction chunks
NCOL = 2048          # minicolumns
NCH = NCOL // 512    # 4 psum column chunks

_BUILD_CACHE = {}


def _build(k_active: int):
    nc = bacc.Bacc("TRN2", target_bir_lowering=False)
    # x passes: [m, ks(128), kc2, pair, tok] ; c6: [ks(128), kc2, pair, col]
    xa = nc.dram_tensor("xa", [M_TILES, 128, KC2 * 2 * 128], FP8, kind="ExternalInput")
    xb = nc.dram_tensor("xb", [M_TILES, 128, KC2 * 2 * 128], FP8, kind="ExternalInput")
    xc = nc.dram_tensor("xc", [M_TILES, 128, KC2 * 2 * 128], FP8, kind="ExternalInput")
    c6 = nc.dram_tensor("c6", [128, KC2, 2, NCOL], FP8, kind="ExternalInput")
    bc = nc.dram_tensor("bc", [128, NCOL], F32, kind="ExternalInput")
    out = nc.dram_tensor("out", [M_TILES, 128, NCOL], BF16, kind="ExternalOutput")

    rounds = max(1, math.ceil(k_active / 8))
    t_idx = (k_active - 1) % 8
    DR = mybir.MatmulPerfMode.DoubleRow

    with tile.TileContext(nc) as tc:
        with (
            tc.tile_pool(name="cpool", bufs=1) as cpool,
            tc.tile_pool(name="xpool", bufs=4) as xpool,
            tc.tile_pool(name="psum", bufs=8, space="PSUM") as pspool,
            tc.tile_pool(name="bpool", bufs=2) as bpool,
            tc.tile_pool(name="wpool", bufs=2) as wpool,
            tc.tile_pool(name="opool", bufs=2) as opool,
        ):
            ct = []

            def load_ct(j):
                # two half DMAs (column halves) so delivery granularity
                # matches the warm-up consumption rate
                t = cpool.tile([128, 2, NCOL], FP8, tag=f"c6_{j}")
                nc.sync.dma_start(t[:, :, :NCOL // 2], c6[:, j, :, :NCOL // 2])
                nc.sync.dma_start(t[:, :, NCOL // 2:], c6[:, j, :, NCOL // 2:])
                ct.append(t)

            def xtile(name, dram, m):
                t = xpool.tile([128, KC2, 2, 128], FP8, tag=name)
                nc.sync.dma_start(t[:], dram[m])
                return t

            # DMA issue order (= serialization order on the DMA engines):
            # the six phase-1 x tiles' FIRST halves (j 0-7) go out up front,
            # interleaved with the first C chunks, so all 8 in-flight blocks
            # (psum bufs) have j-outer work as soon as each C chunk lands;
            # the x second halves follow before j=8 is reached; everything
            # is resident well before the second half of the m-tiles.
            xt = [[None] * 3 for _ in range(M_TILES)]
            XD = {("xa", 0): xa, ("xb", 1): xb, ("xc", 2): xc}
            for (name, pi), dram in XD.items():
                for m in (0, 1):
                    xt[m][pi] = xpool.tile(
                        [128, KC2, 2, 128], FP8, tag=name, name=f"{name}{m}")
            HALF = KC2 // 2 * 2 * 128

            def xhalf(m, pi, h):
                dram = (xa, xb, xc)[pi]
                t = xt[m][pi]
                if h == 0:
                    nc.sync.dma_start(t[:, :KC2 // 2], dram[m][:, :HALF])
                else:
                    nc.sync.dma_start(t[:, KC2 // 2:], dram[m][:, HALF:])

            xhalf(0, 0, 0)
            load_ct(0)
            xhalf(1, 0, 0)
            xhalf(0, 1, 0)
            load_ct(1)
            xhalf(1, 1, 0)
            xhalf(0, 2, 0)
            load_ct(2)
            xhalf(1, 2, 0)
            load_ct(3)
            xhalf(0, 0, 1)
            xhalf(1, 0, 1)
            load_ct(4)
            xhalf(0, 1, 1)
            xhalf(1, 1, 1)
            load_ct(5)
            xhalf(0, 2, 1)
            xhalf(1, 2, 1)
            for j in range(6, KC2):
                load_ct(j)
            bc_t = cpool.tile([128, NCOL], F32)
            nc.sync.dma_start(bc_t[:], bc[:])
            for m in (2, 3):
                xt[m][0] = xtile("xa", xa, m)
                xt[m][1] = xtile("xb", xb, m)
                xt[m][2] = xtile("xc", xc, m)

            def matmuls_for(m, n, ps, j_range, pass_range):
                for pi in pass_range:
                    xp = xt[m][pi]
                    for j in j_range:
                        nc.tensor.matmul(
                            ps[:],
                            xp[:, j, :, :],
                            ct[j][:, :, n * 512:(n + 1) * 512],
                            start=(pi == 0 and j == 0),
                            stop=(pi == 2 and j == KC2 - 1),
                            perf_mode=DR,
                        )

            NB = 8 * rounds          # per-block survivors (40 for k=40)
            FW = 3 * NB + 64         # final round width

            def block_rounds(n, cands, cands2):
                # exact top-NB of block n's 64 candidates -> cands2 slice
                # (runs under the next block's matmuls)
                wcb = wpool.tile([128, 64], F32, tag="wcb")
                src = cands[:, n * 64:(n + 1) * 64]
                for r in range(rounds):
                    m8 = cands2[:, n * NB + r * 8:n * NB + (r + 1) * 8]
                    nc.vector.max(m8, src)
                    if r != rounds - 1:
                        nc.vector.match_replace(wcb[:], m8, src, 0.0)
                        src = wcb[:]

            def finish_block(m, n, ps, boosted, cands, cands2):
                # cands2 set (last m-tile): blocks 0..2 reduce to their exact
                # top-NB under the matmuls and the last block's raw segment
                # maxes land directly in the final array, shortening the
                # critical tail chain. cands2 None: plain 256-wide candidates.
                blk = boosted[:, n * 512:(n + 1) * 512]
                nc.vector.tensor_tensor(
                    blk, ps[:], bc_t[:, n * 512:(n + 1) * 512],
                    mybir.AluOpType.mult)
                if k_active <= 48:
                    for s in range(8):
                        dst = (cands[:, (n * 8 + s) * 8:(n * 8 + s + 1) * 8]
                               if (cands2 is None or n < NCH - 1) else
                               cands2[:, 3 * NB + s * 8:3 * NB + (s + 1) * 8])
                        nc.vector.max(
                            dst,
                            boosted[:, (n * 8 + s) * 64:(n * 8 + s + 1) * 64],
                        )
                    if cands2 is not None and n < NCH - 1:
                        block_rounds(n, cands, cands2)

            # Phase 1 (m0+m1, all 8 psum banks): j-outer emission so every
            # arriving C chunk immediately feeds all 8 in-flight blocks.
            row = {}
            for m in (0, 1):
                row[m] = (bpool.tile([128, NCOL], F32, tag="boosted",
                                     name=f"boosted{m}"),
                          wpool.tile([128, 32 * 8], F32, tag="cands",
                                     name=f"cands{m}"),
                          None)
            ps1 = {(m, n): pspool.tile([128, 512], F32, tag="ps",
                                       name=f"ps{m}{n}")
                   for m in (0, 1) for n in range(NCH)}
            # j-outer only over the C-arrival window; then complete
            # blocks one at a time (C resident by then) so they stop
            # staggered and the DVE chains start ~25us in rather than at
            # phase-1's end.
            JW = 9
            for j in range(JW):
                for pi in range(3):
                    for m in (0, 1):
                        for n in range(NCH):
                            matmuls_for(m, n, ps1[(m, n)], [j], [pi])

            def tail_chain(m, boosted, src_c, width):
                if k_active <= 48:
                    # Exact k-th largest of the surviving candidates (a
                    # 64-col segment contributes >8 of the top-k with prob
                    # ~2e-4 per row for k=40), then threshold-mask the row.
                    tops = wpool.tile([128, 8 * rounds], F32, tag="tops")
                    wc = wpool.tile([128, width], F32, tag="wc")
                    src = src_c[:, :width]
                    for r in range(rounds):
                        m8 = tops[:, r * 8:(r + 1) * 8]
                        nc.vector.max(m8, src)
                        if r != rounds - 1:
                            nc.vector.match_replace(wc[:], m8, src, 0.0)
                            src = wc[:]
                    thr = tops[:, (rounds - 1) * 8 + t_idx:
                               (rounds - 1) * 8 + t_idx + 1]
                    # masked = (boosted >= thr) * boosted, fused, in quarters
                    # with the output DMA per quarter (issued from the DVE
                    # queue: no cross-engine hop) to keep the tail short.
                    mbf = opool.tile([128, NCOL], BF16, tag="mbf")
                    edges = (0, 1024, 1536, 1792, 2048)
                    for h in range(4):
                        sl = slice(edges[h], edges[h + 1])
                        nc.vector.scalar_tensor_tensor(
                            mbf[:, sl], boosted[:, sl], thr, boosted[:, sl],
                            mybir.AluOpType.is_ge, mybir.AluOpType.mult)
                        nc.scalar.dma_start(out[m][:, sl], mbf[:, sl])
                else:
                    # Exact full-width chain: zero the top-k in a working
                    # copy, then masked = boosted - working.
                    rem = k_active % 8
                    tops = wpool.tile([128, 8 * rounds], F32, tag="tops")
                    w = wpool.tile([128, NCOL], F32, tag="w")
                    src = boosted
                    for r in range(rounds):
                        m8 = tops[:, r * 8:(r + 1) * 8]
                        nc.vector.max(m8, src[:])
                        if r == rounds - 1 and rem:
                            nc.gpsimd.memset(m8[:, rem:], -1e30)
                        nc.vector.match_replace(w[:], m8, src[:], 0.0)
                        src = w
                    mbf = opool.tile([128, NCOL], BF16, tag="mbf")
                    nc.vector.tensor_tensor(
                        mbf[:], boosted[:], w[:], mybir.AluOpType.subtract)
                    nc.sync.dma_start(out[m], mbf[:])

            for m in (0, 1):
                for n in range(NCH):
                    matmuls_for(m, n, ps1[(m, n)], range(JW, KC2), range(3))
                    finish_block(m, n, ps1[(m, n)], *row[m])
                tail_chain(m, row[m][0], row[m][1], 256)

            # Phase 2 (m2, m3): C fully resident — block-sequential.
            # Only the final m-tile uses the per-block top-NB reduction
            # (shorter tail); earlier tiles keep the cheaper 256-wide rounds.
            for m in (2, 3):
                boosted = bpool.tile([128, NCOL], F32, tag="boosted")
                cands = wpool.tile([128, 32 * 8], F32, tag="cands")
                cands2 = (wpool.tile([128, FW], F32, tag="cands2",
                                     name="cands2")
                          if m == 3 else None)
                for n in range(NCH):
                    ps = pspool.tile([128, 512], F32, tag="ps")
                    matmuls_for(m, n, ps, range(KC2), range(3))
                    finish_block(m, n, ps, boosted, cands, cands2)
                if m == 3:
                    tail_chain(m, boosted, cands2, FW)
                else:
                    tail_chain(m, boosted, cands, 256)
    nc.compile()
    return nc


def _get_nc(k_active: int):
    nc = _BUILD_CACHE.get(k_active)
    if nc is None:
        nc = _BUILD_CACHE[k_active] = _build(k_active)
    return nc


def _fp8_split3(x):
    """x (f32, [0,1)) -> (a, b, c) e4m3 with (a + b + c)/64 ~ x
    (residual <= 2^-15)."""
    a = (x * 64.0).astype(E4)
    r1 = x * 64.0 - a.astype(np.float32)
    b = r1.astype(E4)
    r2 = r1 - b.astype(np.float32)
    c = r2.astype(E4)
    return a, b, c


def kernel(input_vector, connections, boosting_factors, num_active):
    x = np.ascontiguousarray(input_vector, dtype=np.float32).reshape(-1, D)
    b = np.ascontiguousarray(boosting_factors, dtype=np.float32)
    k = min(int(num_active), NCOL)
    n_tok = x.shape[0]
    assert n_tok == N_CORES * TOK_PER_CORE, n_tok

    nc = _get_nc(k)

    # x^T laid out as [core, m, ks(part), kc2, pair, tok]
    xt = np.ascontiguousarray(x.T)                         # [D, n_tok]
    xt = xt.reshape(KC2, 2, 128, N_CORES, M_TILES, 128)    # [j, i, ks, core, m, t]
    xt = xt.transpose(3, 4, 2, 0, 1, 5)                    # [core, m, ks, j, i, t]
    xt = np.ascontiguousarray(xt).reshape(N_CORES, M_TILES, 128, KC2 * 2 * 128)
    xa, xb, xc = _fp8_split3(xt)

    # C^T laid out as [ks(part), kc2, pair, col]; {0, 2^-6} exact in e4m3
    ct = np.ascontiguousarray(connections.T, dtype=np.float32)  # [D, NCOL]
    ct = ct.reshape(KC2, 2, 128, NCOL).transpose(2, 0, 1, 3)
    c6 = (np.ascontiguousarray(ct) * 0.015625).astype(E4)

    bcast = np.ascontiguousarray(np.broadcast_to(b, (128, NCOL)))

    in_maps = [
        {"xa": xa[cidx], "xb": xb[cidx], "xc": xc[cidx], "c6": c6, "bc": bcast}
        for cidx in range(N_CORES)
    ]
    res = run_bass_kernel_spmd(nc, in_maps, core_ids=list(range(N_CORES)))
    outs = [r["out"].astype(np.float32).reshape(TOK_PER_CORE, NCOL)
            for r in res.results]
    full = np.concatenate(outs, axis=0)
    return full.reshape(input_vector.shape[0], input_vector.shape[1], NCOL)


# revision 26
# speedup vs baseline: 1.0163x; 1.0035x over previous
"""HTM spatial-pooler kernel for Trainium2 (8 NeuronCores, data-parallel over tokens).

Computes, for x = input_vector reshaped to [4096 tokens, 4096]:
    overlap = x @ C^T               (C = connections [2048, 4096], binary)
    boosted = overlap * boost       (per-column boosting factors)
    masked  = where(boosted >= kth_largest_per_row(boosted, k), boosted, 0)

Strategy per core (512 tokens):
  - Matmul as THREE fp8(e4m3) passes in DoubleRow perf mode (0.5 cycles/row,
    2 contraction sub-tiles per instruction), all accumulating into a single
    PSUM bank per 512-column block. Scale alignment is folded into a single
    resident copy of C at scale 2^-6 (values {0, 2^-6}, exact in e4m3):
        64*x ~ a + b + c,   overlap = (a+b+c) @ (C * 2^-6)
    with a = e4m3(64x), b = e4m3(64x - a), c = e4m3(64x - a - b). Because
    the e4m3 subnormal floor (2^-9) is divided by the C scale, the residual
    is <= 2^-15 in x units — the top-k mask matches the exact fp32 mask
    except for genuinely tied rows, with no DVE combine passes needed.
  - DVE applies boosting per block, then computes the per-row k-th-largest
    via segmented max8/match_replace and masks with a fused
    (boosted >= thr) * boosted scalar_tensor_tensor. Output stored as bf16.
"""
import math

import numpy as np
import ml_dtypes

import concourse.bacc as bacc
import concourse.mybir as mybir
from concourse import tile
from concourse.bass_utils import run_bass_kernel_spmd

FP8 = mybir.dt.float8e4
BF16 = mybir.dt.bfloat16
F32 = mybir.dt.float32
E4 = ml_dtypes.float8_e4m3

N_CORES = 8
TOK_PER_CORE = 512
M_TILES = 4          # 128-token tiles per core
D = 4096             # input size (contraction)
KC2 = D // 256       # 16 double-row contraction chunks
NCOL = 2048          # minicolumns
NCH = NCOL // 512    # 4 psum column chunks

_BUILD_CACHE = {}


def _build(k_active: int):
    nc = bacc.Bacc("TRN2", target_bir_lowering=False)
    # x passes: [m, ks(128), kc2, pair, tok] ; c6: [ks(128), kc2, pair, col]
    xa = nc.dram_tensor("xa", [M_TILES, 128, KC2 * 2 * 128], FP8, kind="ExternalInput")
    xb = nc.dram_tensor("xb", [M_TILES, 128, KC2 * 2 * 128], FP8, kind="ExternalInput")
    xc = nc.dram_tensor("xc", [M_TILES, 128, KC2 * 2 * 128], FP8, kind="ExternalInput")
    c6 = nc.dram_tensor("c6", [128, KC2, 2, NCOL], FP8, kind="ExternalInput")
    bc = nc.dram_tensor("bc", [128, NCOL], F32, kind="ExternalInput")
    out = nc.dram_tensor("out", [M_TILES, 128, NCOL], BF16, kind="ExternalOutput")

    rounds = max(1, math.ceil(k_active / 8))
    t_idx = (k_active - 1) % 8
    DR = mybir.MatmulPerfMode.DoubleRow

    with tile.TileContext(nc) as tc:
        with (
            tc.tile_pool(name="cpool", bufs=1) as cpool,
            tc.tile_pool(name="xpool", bufs=4) as xpool,
            tc.tile_pool(name="psum", bufs=8, space="PSUM") as pspool,
            tc.tile_pool(name="bpool", bufs=2) as bpool,
            tc.tile_pool(name="wpool", bufs=2) as wpool,
            tc.tile_pool(name="opool", bufs=2) as opool,
        ):
            ct = []

            def load_ct(j):
                # two half DMAs (column halves) so delivery granularity
                # matches the warm-up consumption rate
                t = cpool.tile([128, 2, NCOL], FP8, tag=f"c6_{j}")
                nc.sync.dma_start(t[:, :, :NCOL // 2], c6[:, j, :, :NCOL // 2])
                nc.sync.dma_start(t[:, :, NCOL // 2:], c6[:, j, :, NCOL // 2:])
                ct.append(t)

            def xtile(name, dram, m):
                t = xpool.tile([128, KC2, 2, 128], FP8, tag=name)
                nc.sync.dma_start(t[:], dram[m])
                return t

            # DMA issue order (= serialization order on the DMA engines):
            # the six phase-1 x tiles' FIRST halves (j 0-7) go out up front,
            # interleaved with the first C chunks, so all 8 in-flight blocks
            # (psum bufs) have j-outer work as soon as each C chunk lands;
            # the x second halves follow before j=8 is reached; everything
            # is resident well before the second half of the m-tiles.
            xt = [[None] * 3 for _ in range(M_TILES)]
            XD = {("xa", 0): xa, ("xb", 1): xb, ("xc", 2): xc}
            for (name, pi), dram in XD.items():
                for m in (0, 1):
                    xt[m][pi] = xpool.tile(
                        [128, KC2, 2, 128], FP8, tag=name, name=f"{name}{m}")
            HALF = KC2 // 2 * 2 * 128

            def xhalf(m, pi, h):
                dram = (xa, xb, xc)[pi]
                t = xt[m][pi]
                if h == 0:
                    nc.sync.dma_start(t[:, :KC2 // 2], dram[m][:, :HALF])
                else:
                    nc.sync.dma_start(t[:, KC2 // 2:], dram[m][:, HALF:])

            xhalf(0, 0, 0)
            load_ct(0)
            xhalf(1, 0, 0)
            xhalf(0, 1, 0)
            load_ct(1)
            xhalf(1, 1, 0)
            xhalf(0, 2, 0)
            load_ct(2)
            xhalf(1, 2, 0)
            load_ct(3)
            xhalf(0, 0, 1)
            xhalf(1, 0, 1)
            load_ct(4)
            xhalf(0, 1, 1)
            xhalf(1, 1, 1)
            load_ct(5)
            xhalf(0, 2, 1)
            xhalf(1, 2, 1)
            for j in range(6, KC2):
                load_ct(j)
            bc_t = cpool.tile([128, NCOL], F32)
            nc.sync.dma_start(bc_t[:], bc[:])
            for m in (2, 3):
                xt[m][0] = xtile("xa", xa, m)
                xt[m][1] = xtile("xb", xb, m)
                xt[m][2] = xtile("xc", xc, m)

            def matmuls_for(m, n, ps, j_range, pass_range):
                for pi in pass_range:
                    xp = xt[m][pi]
                    for j in j_range:
                        nc.tensor.matmul(
                            ps[:],
                            xp[:, j, :, :],
                            ct[j][:, :, n * 512:(n + 1) * 512],
                            start=(pi == 0 and j == 0),
                            stop=(pi == 2 and j == KC2 - 1),
                            perf_mode=DR,
                        )

            NB = 8 * rounds          # per-block survivors (40 for k=40)
            FW = 3 * NB + 64         # final round width

            def block_rounds(n, cands, cands2):
                # exact top-NB of block n's 64 candidates -> cands2 slice
                # (runs under the next block's matmuls)
                wcb = wpool.tile([128, 64], F32, tag="wcb")
                src = cands[:, n * 64:(n + 1) * 64]
                for r in range(rounds):
                    m8 = cands2[:, n * NB + r * 8:n * NB + (r + 1) * 8]
                    nc.vector.max(m8, src)
                    if r != rounds - 1:
                        nc.vector.match_replace(wcb[:], m8, src, 0.0)
                        src = wcb[:]

            def finish_block(m, n, ps, boosted, cands, cands2):
                # cands2 set (last m-tile): blocks 0..2 reduce to their exact
                # top-NB under the matmuls and the last block's raw segment
                # maxes land directly in the final array, shortening the
                # critical tail chain. cands2 None: plain 256-wide candidates.
                blk = boosted[:, n * 512:(n + 1) * 512]
                nc.vector.tensor_tensor(
                    blk, ps[:], bc_t[:, n * 512:(n + 1) * 512],
                    mybir.AluOpType.mult)
                if k_active <= 48:
                    for s in range(8):
                        dst = (cands[:, (n * 8 + s) * 8:(n * 8 + s + 1) * 8]
                               if (cands2 is None or n < NCH - 1) else
                               cands2[:, 3 * NB + s * 8:3 * NB + (s + 1) * 8])
                        nc.vector.max(
                            dst,
                            boosted[:, (n * 8 + s) * 64:(n * 8 + s + 1) * 64],
                        )
                    if cands2 is not None and n < NCH - 1:
                        block_rounds(n, cands, cands2)

            # Phase 1 (m0+m1, all 8 psum banks): j-outer emission so every
            # arriving C chunk immediately feeds all 8 in-flight blocks.
            row = {}
            for m in (0, 1):
                row[m] = (bpool.tile([128, NCOL], F32, tag="boosted",
                                     name=f"boosted{m}"),
                          wpool.tile([128, 32 * 8], F32, tag="cands",
                                     name=f"cands{m}"),
                          None)
            ps1 = {(m, n): pspool.tile([128, 512], F32, tag="ps",
                                       name=f"ps{m}{n}")
                   for m in (0, 1) for n in range(NCH)}
            # j-outer only over the C-arrival window; then complete
            # blocks one at a time (C resident by then) so they stop
            # staggered and the DVE chains start ~25us in rather than at
            # phase-1's end.
            JW = 9
            for j in range(JW):
                for pi in range(3):
                    for m in (0, 1):
                        for n in range(NCH):
                            matmuls_for(m, n, ps1[(m, n)], [j], [pi])

            def tail_chain(m, boosted, src_c, width):
                if k_active <= 48:
                    # Exact k-th largest of the surviving candidates (a
                    # 64-col segment contributes >8 of the top-k with prob
                    # ~2e-4 per row for k=40), then threshold-mask the row.
                    tops = wpool.tile([128, 8 * rounds], F32, tag="tops")
                    wc = wpool.tile([128, width], F32, tag="wc")
                    src = src_c[:, :width]
                    for r in range(rounds):
                        m8 = tops[:, r * 8:(r + 1) * 8]
                        nc.vector.max(m8, src)
                        if r != rounds - 1:
                            nc.vector.match_replace(wc[:], m8, src, 0.0)
                            src = wc[:]
                    thr = tops[:, (rounds - 1) * 8 + t_idx:
                               (rounds - 1) * 8 + t_idx + 1]
                    # masked = (boosted >= thr) * boosted, fused, in quarters
                    # with the output DMA per quarter (issued from the DVE
                    # queue: no cross-engine hop) to keep the tail short.
                    mbf = opool.tile([128, NCOL], BF16, tag="mbf")
                    edges = (0, 512, 1024, 1536, 2048)
                    for h in range(4):
                        sl = slice(edges[h], edges[h + 1])
                        nc.vector.scalar_tensor_tensor(
                            mbf[:, sl], boosted[:, sl], thr, boosted[:, sl],
                            mybir.AluOpType.is_ge, mybir.AluOpType.mult)
                        nc.scalar.dma_start(out[m][:, sl], mbf[:, sl])
                else:
                    # Exact full-width chain: zero the top-k in a working
                    # copy, then masked = boosted - working.
                    rem = k_active % 8
                    tops = wpool.tile([128, 8 * rounds], F32, tag="tops")
                    w = wpool.tile([128, NCOL], F32, tag="w")
                    src = boosted
                    for r in range(rounds):
                        m8 = tops[:, r * 8:(r + 1) * 8]
                        nc.vector.max(m8, src[:])
                        if r == rounds - 1 and rem:
                            nc.gpsimd.memset(m8[:, rem:], -1e30)
                        nc.vector.match_replace(w[:], m8, src[:], 0.0)
                        src = w
                    mbf = opool.tile([128, NCOL], BF16, tag="mbf")
                    nc.vector.tensor_tensor(
                        mbf[:], boosted[:], w[:], mybir.AluOpType.subtract)
                    nc.sync.dma_start(out[m], mbf[:])

            for m in (0, 1):
                for n in range(NCH):
                    matmuls_for(m, n, ps1[(m, n)], range(JW, KC2), range(3))
                    finish_block(m, n, ps1[(m, n)], *row[m])
                tail_chain(m, row[m][0], row[m][1], 256)

            # Phase 2 (m2, m3): C fully resident — block-sequential.
            # Only the final m-tile uses the per-block top-NB reduction
            # (shorter tail); earlier tiles keep the cheaper 256-wide rounds.
            for m in (2, 3):
                boosted = bpool.tile([128, NCOL], F32, tag="boosted")
                cands = wpool.tile([128, 32 * 8], F32, tag="cands")
                cands2 = (wpool.tile([128, FW], F32, tag="cands2",
                                     name="cands2")
                          if m == 3 else None)
                for n in range(NCH):
                    ps = pspool.tile([128, 512], F32, tag="ps")
                    matmuls_for(m, n, ps, range(KC2), range(3))
                    finish_block(m, n, ps, boosted, cands, cands2)
                if m == 3:
                    tail_chain(m, boosted, cands2, FW)
                else:
                    tail_chain(m, boosted, cands, 256)
    nc.compile()
    return nc


def _get_nc(k_active: int):
    nc = _BUILD_CACHE.get(k_active)
    if nc is None:
        nc = _BUILD_CACHE[k_active] = _build(k_active)
    return nc


def _fp8_split3(x):
    """x (f32, [0,1)) -> (a, b, c) e4m3 with (a + b + c)/64 ~ x
    (residual <= 2^-15)."""
    a = (x * 64.0).astype(E4)
    r1 = x * 64.0 - a.astype(np.float32)
    b = r1.astype(E4)
    r2 = r1 - b.astype(np.float32)
    c = r2.astype(E4)
    return a, b, c


def kernel(input_vector, connections, boosting_factors, num_active):
    x = np.ascontiguousarray(input_vector, dtype=np.float32).reshape(-1, D)
    b = np.ascontiguousarray(boosting_factors, dtype=np.float32)
    k = min(int(num_active), NCOL)
    n_tok = x.shape[0]
    assert n_tok == N_CORES * TOK_PER_CORE, n_tok

    nc = _get_nc(k)

    # x^T laid out as [core, m, ks(part), kc2, pair, tok]
    xt = np.ascontiguousarray(x.T)                         # [D, n_tok]
    xt = xt.reshape(KC2, 2, 128, N_CORES, M_TILES, 128)    # [j, i, ks, core, m, t]
    xt = xt.transpose(3, 4, 2, 0, 1, 5)                    # [core, m, ks, j, i, t]
    xt = np.ascontiguousarray(xt).reshape(N_CORES, M_TILES, 128, KC2 * 2 * 128)
    xa, xb, xc = _fp8_split3(xt)

    # C^T laid out as [ks(part), kc2, pair, col]; {0, 2^-6} exact in e4m3
    ct = np.ascontiguousarray(connections.T, dtype=np.float32)  # [D, NCOL]
    ct = ct.reshape(KC2, 2, 128, NCOL).transpose(2, 0, 1, 3)
    c6 = (np.ascontiguousarray(ct) * 0.015625).astype(E4)

    bcast = np.ascontiguousarray(np.broadcast_to(b, (128, NCOL)))

    in_maps = [
        {"xa": xa[cidx], "xb": xb[cidx], "xc": xc[cidx], "c6": c6, "bc": bcast}
        for cidx in range(N_CORES)
    ]
    res = run_bass_kernel_spmd(nc, in_maps, core_ids=list(range(N_CORES)))
    outs = [r["out"].astype(np.float32).reshape(TOK_PER_CORE, NCOL)
            for r in res.results]
    full = np.concatenate(outs, axis=0)
    return full.reshape(input_vector.shape[0], input_vector.shape[1], NCOL)
